# revision 3
# baseline (speedup 1.0000x reference)
"""Trainium2 Bass kernel for nn_CrossAttention (B=8, L=K=512, M=N=P=D=64).

Math per batch b (one batch per NeuronCore, 8 cores):
  scoresT[k,l] = scale * (K @ Q^T)            # PE, contract D=64
  ET = exp(scoresT)                           # ACT (softmax w/o max-sub: |s|<~45 safe in f32)
  sums[l] = colsum_k ET                       # PE ones-matmul
  vkc[k,n] = sum_p vk[k,p,n]*vexp[k,p]        # DVE mult(+bcast) + reduce
  tmpT[n,l] = vkc^T @ ET                      # PE, contract k
  tmp[l,n]  = transpose(tmpT)                 # PE transpose
  attn[l,m] = sum_n vq[l,m,n]*tmp[l,n]        # DVE mult(+bcast) + reduce
  x = attn/sums + q ; out = LN(x)*gamma+beta  # DVE/ACT
"""

import numpy as np

B = 8
L = 512
KK = 512
MM = 64
NN = 64
PP = 64
DD = 64
NCORES = 8

# bf16 value path: vq cast to bf16 during DMA, products kept in bf16, and the
# p/n reductions done as bf16 tree-adds (DVE 2x mode) instead of 1x
# tensor_reduce. Softmax, scores, tmpT, residual and LayerNorm stay f32.
BF16_VALUE = True

_CACHE = {}


def _patch_multiwait_split():
    """This environment's walrus accepts only ONE sem-wait per instruction,
    while Tile emits instructions carrying several. Rewrite the BIR JSON just
    before compilation: hoist excess waits onto single-wait NoOps inserted
    immediately before the offending instruction on the same engine."""
    import json

    from concourse import bass_utils, bass2jax

    if getattr(bass_utils, "_multiwait_split_patched", False):
        return

    orig = bass_utils.compile_bir_kernel

    def _split(bir_json):
        if isinstance(bir_json, bytes):
            m = json.loads(bir_json.decode())
        else:
            m = json.loads(bir_json)
        cnt = 0
        for fn in m["functions"]:
            for bb in fn["blocks"]:
                insts = bb["instructions"]
                out = []
                for inst in insts:
                    si = inst.get("sync_info")
                    waits = si.get("on_wait", []) if si else []
                    if len(waits) > 1:
                        for w in waits[:-1]:
                            cnt += 1
                            out.append(
                                {
                                    "name": f"WS-{cnt}-{inst['name']}",
                                    "opcode": "NoOp",
                                    "engine": inst["engine"],
                                    "ins": [],
                                    "outs": [],
                                    "debug": inst.get("debug", 0),
                                    "sync_info": {
                                        "on_update": [],
                                        "on_wait": [w],
                                    },
                                }
                            )
                        si["on_wait"] = [waits[-1]]
                    out.append(inst)
                bb["instructions"] = out
        return json.dumps(m).encode()

    def patched(bir_json, tmpdir, neff_name="file.neff", **kw):
        return orig(_split(bir_json), tmpdir, neff_name=neff_name, **kw)

    bass_utils.compile_bir_kernel = patched
    bass2jax.compile_bir_kernel = patched
    bass_utils._multiwait_split_patched = True


def _build_nc(loop_n=0):
    import concourse.bass as bass
    import concourse.tile as tile
    from concourse import mybir
    from concourse.masks import make_identity

    _patch_multiwait_split()

    f32 = mybir.dt.float32
    vdt = mybir.dt.bfloat16 if BF16_VALUE else f32
    Alu = mybir.AluOpType
    Act = mybir.ActivationFunctionType

    nc = bass.Bass()
    q_d = nc.dram_tensor("q", [L, DD], f32, kind="ExternalInput")
    k_d = nc.dram_tensor("k", [KK, DD], f32, kind="ExternalInput")
    vq_d = nc.dram_tensor("vq", [L, MM * NN], f32, kind="ExternalInput")
    vk_d = nc.dram_tensor("vk", [KK, PP * NN], f32, kind="ExternalInput")
    vexp_d = nc.dram_tensor("vexp", [KK, PP], f32, kind="ExternalInput")
    scale_d = nc.dram_tensor("scale", [1, 1], f32, kind="ExternalInput")
    gamma_d = nc.dram_tensor("ln_gamma", [1, DD], f32, kind="ExternalInput")
    beta_d = nc.dram_tensor("ln_beta", [1, DD], f32, kind="ExternalInput")
    out_d = nc.dram_tensor("out", [L, MM], f32, kind="ExternalOutput")

    LT = L // 128   # 4 l-tiles
    KT = KK // 128  # 4 k-tiles

    with tile.TileContext(nc) as tc:
        import contextlib

        loop_cm = tc.For_i(0, loop_n, 1) if loop_n else contextlib.nullcontext()
        lp_cm = nc.allow_low_precision("bf16 value-path partial sums")
        with loop_cm, lp_cm, contextlib.ExitStack() as ctx:
            const = ctx.enter_context(tc.tile_pool(name="const", bufs=1))
            vk_pool = ctx.enter_context(tc.tile_pool(name="vk", bufs=4))
            vq_pool = ctx.enter_context(tc.tile_pool(name="vq", bufs=2))
            prod_pool = ctx.enter_context(tc.tile_pool(name="prod", bufs=2))
            small = ctx.enter_context(tc.tile_pool(name="small", bufs=2))
            ps_scores = ctx.enter_context(
                tc.tile_pool(name="ps_s", bufs=2, space="PSUM")
            )
            ps_acc = ctx.enter_context(tc.tile_pool(name="ps_acc", bufs=1, space="PSUM"))
            ps_tr = ctx.enter_context(tc.tile_pool(name="ps_tr", bufs=2, space="PSUM"))

            # ---- first vk half-tile DMA issued before everything else so the
            # DVE can start step A as early as possible ----
            HALF = PP // 2  # split each [128, 64, 64] vk tile into two p-halves
            vk_halves = []
            vkh = vk_pool.tile([128, HALF, NN], f32, tag="vk")
            nc.sync.dma_start(out=vkh, in_=vk_d[0:128, 0 : HALF * NN])
            vk_halves.append(vkh)

            vexp_nat = const.tile([128, KT, PP], f32)
            nc.sync.dma_start(
                out=vexp_nat, in_=vexp_d[:].rearrange("(t p) d -> p t d", p=128)
            )
            q_nat = const.tile([128, LT, DD], f32)
            nc.sync.dma_start(out=q_nat, in_=q_d[:].rearrange("(t p) d -> p t d", p=128))
            k_nat = const.tile([128, KT, DD], f32)
            nc.sync.dma_start(out=k_nat, in_=k_d[:].rearrange("(t p) d -> p t d", p=128))

            # ---- constants ----
            identity = const.tile([128, 128], f32)
            make_identity(nc, identity)
            ones_col = const.tile([128, 1], f32)
            nc.vector.memset(ones_col, 1.0)
            scale_bc = const.tile([128, 1], f32)
            nc.sync.dma_start(out=scale_bc, in_=scale_d[:].to_broadcast([128, 1]))
            gamma_bc = const.tile([128, DD], f32)
            nc.sync.dma_start(out=gamma_bc, in_=gamma_d[:].to_broadcast([128, DD]))
            beta_bc = const.tile([128, DD], f32)
            nc.sync.dma_start(out=beta_bc, in_=beta_d[:].to_broadcast([128, DD]))
            zero_t = const.tile([128, 1], f32)
            nc.vector.memset(zero_t, 0.0)
            eps_t = const.tile([128, 1], f32)
            nc.vector.memset(eps_t, 1e-3)

            # remaining vk half-tiles
            for h in range(1, KT * 2):
                i, hh = divmod(h, 2)
                vkh = vk_pool.tile([128, HALF, NN], f32, tag="vk")
                nc.sync.dma_start(
                    out=vkh,
                    in_=vk_d[i * 128 : (i + 1) * 128, hh * HALF * NN : (hh + 1) * HALF * NN],
                )
                vk_halves.append(vkh)

            # ---- qT, kT via PE transpose ----
            qT = const.tile([64, L], f32)
            kT = const.tile([64, KK], f32)
            for i in range(LT):
                pq = ps_tr.tile([64, 128], f32, tag="tr")
                nc.tensor.transpose(pq, q_nat[:, i, :], identity)
                nc.scalar.copy(qT[:, i * 128 : (i + 1) * 128], pq)
            for i in range(KT):
                pk = ps_tr.tile([64, 128], f32, tag="tr")
                nc.tensor.transpose(pk, k_nat[:, i, :], identity)
                nc.scalar.copy(kT[:, i * 128 : (i + 1) * 128], pk)

            # ---- scoresT -> ET ; step A (vkc) over half-tiles ----
            ET = const.tile([128, KT, L], f32)
            vkc = const.tile([128, KT, NN], f32)
            for i in range(KT):
                ps_s = ps_scores.tile([128, L], f32, tag="sc")
                nc.tensor.matmul(
                    ps_s, lhsT=kT[:, i * 128 : (i + 1) * 128], rhs=qT[:],
                    start=True, stop=True,
                )
                nc.scalar.activation(
                    ET[:, i, :], ps_s, func=Act.Exp, bias=zero_t[:], scale=scale_bc[:],
                )

                part = small.tile([128, 2, NN], f32, tag="part")
                for hh in range(2):
                    vkh = vk_halves[2 * i + hh]
                    pr = prod_pool.tile([128, HALF, NN], vdt, tag="prod")
                    nc.vector.tensor_tensor(
                        pr[:],
                        vkh[:],
                        vexp_nat[
                            :, i, hh * HALF : (hh + 1) * HALF, None
                        ].to_broadcast([128, HALF, NN]),
                        Alu.mult,
                    )
                    if BF16_VALUE:
                        # tree-reduce over p (outer axis): bf16 adds run at 2x
                        cur = pr
                        w = HALF // 2
                        while w >= 1:
                            if w == 1:
                                nxt = part[:, hh : hh + 1, :]
                            else:
                                nxt = prod_pool.tile(
                                    [128, w, NN], vdt, tag=f"atree{w}"
                                )
                            nc.vector.tensor_tensor(
                                nxt[:], cur[:, 0:w, :], cur[:, w : 2 * w, :],
                                Alu.add,
                            )
                            cur = nxt
                            w //= 2
                    else:
                        nc.vector.reduce_sum(
                            part[:, hh, :], pr[:].rearrange("a p n -> a n p"),
                            axis=mybir.AxisListType.X,
                        )
                nc.vector.tensor_tensor(
                    vkc[:, i, :], part[:, 0, :], part[:, 1, :], Alu.add
                )

            # ---- colsums of ET (softmax denominators), [1, 512] ----
            ps_sum = ps_acc.tile([1, L], f32, tag="sum")
            for i in range(KT):
                nc.tensor.matmul(
                    ps_sum, lhsT=ones_col[:], rhs=ET[:, i, :],
                    start=(i == 0), stop=(i == KT - 1),
                )
            sums_sb = const.tile([1, L], f32)
            nc.scalar.copy(sums_sb, ps_sum)

            # transpose sums [1,512] -> [128,4] and take reciprocal
            ps_sT = ps_tr.tile([128, LT], f32, tag="tr")
            for j in range(LT):
                nc.tensor.transpose(
                    ps_sT[:, j : j + 1], sums_sb[:, j * 128 : (j + 1) * 128],
                    identity[:1, :1],
                )
            recip_col = const.tile([128, LT], f32)
            nc.vector.reciprocal(recip_col, ps_sT)

            # ---- tmpT[n, l] = vkc^T @ ET  (accumulate over k-tiles) ----
            ps_tmpT = ps_acc.tile([64, L], f32, tag="tmpT")
            for i in range(KT):
                nc.tensor.matmul(
                    ps_tmpT, lhsT=vkc[:, i, :], rhs=ET[:, i, :],
                    start=(i == 0), stop=(i == KT - 1),
                )
            tmpT_sb = const.tile([64, L], f32)
            nc.scalar.copy(tmpT_sb, ps_tmpT)

            # transpose tmpT -> tmp (all l-tiles up front, PE + ACT)
            tmp_all = const.tile([128, LT, NN], vdt)
            for j in range(LT):
                ptj = ps_tr.tile([128, NN], f32, tag="tr")
                nc.tensor.transpose(
                    ptj, tmpT_sb[:, j * 128 : (j + 1) * 128], identity[:64, :64]
                )
                nc.scalar.copy(tmp_all[:, j, :], ptj)

            # ---- per l-tile: step C, rescale+residual, LayerNorm ----
            for j in range(LT):
                vq_t = vq_pool.tile([128, MM, NN], vdt, tag="vq")
                if BF16_VALUE:
                    # SWDGE casts f32 -> bf16 in flight
                    nc.gpsimd.dma_start(
                        out=vq_t, in_=vq_d[j * 128 : (j + 1) * 128, :]
                    )
                else:
                    nc.sync.dma_start(out=vq_t, in_=vq_d[j * 128 : (j + 1) * 128, :])
                pr2 = prod_pool.tile([128, MM, NN], vdt, tag="prod2")
                nc.vector.tensor_tensor(
                    pr2[:],
                    vq_t[:],
                    tmp_all[:, j, None, :].to_broadcast([128, MM, NN]),
                    Alu.mult,
                )
                attn = small.tile([128, MM], f32, tag="attn")
                if BF16_VALUE:
                    # tree-reduce over n (inner axis): bf16 adds at 2x
                    cur = pr2
                    w = NN // 2
                    while w >= 1:
                        if w == 1:
                            nxt = attn[:, :, None]
                        else:
                            nxt = prod_pool.tile([128, MM, w], vdt, tag=f"ctree{w}")
                        nc.vector.tensor_tensor(
                            nxt[:], cur[:, :, 0:w], cur[:, :, w : 2 * w], Alu.add
                        )
                        cur = nxt
                        w //= 2
                else:
                    nc.vector.reduce_sum(attn, pr2[:], axis=mybir.AxisListType.X)

                # x = attn * (1/sums) + q
                x = small.tile([128, MM], f32, tag="x")
                nc.vector.scalar_tensor_tensor(
                    out=x, in0=attn, scalar=recip_col[:, j : j + 1],
                    in1=q_nat[:, j, :], op0=Alu.mult, op1=Alu.add,
                )

                # LayerNorm(eps=1e-3)
                stats = small.tile([128, 6], f32, tag="stats")
                nc.vector.bn_stats(out=stats, in_=x[:])
                mv = small.tile([128, 2], f32, tag="mv")
                nc.vector.bn_aggr(out=mv, in_=stats[:])
                sd = small.tile([128, 1], f32, tag="sd")
                nc.scalar.activation(
                    sd, mv[:, 1:2], func=Act.Sqrt, bias=eps_t[:], scale=1.0
                )
                rstd = small.tile([128, 1], f32, tag="rstd")
                nc.vector.reciprocal(rstd, sd)
                xn = small.tile([128, MM], f32, tag="xn")
                nc.vector.tensor_scalar(
                    out=xn, in0=x, scalar1=mv[:, 0:1], scalar2=rstd,
                    op0=Alu.subtract, op1=Alu.mult,
                )
                xg = small.tile([128, MM], f32, tag="xg")
                nc.vector.tensor_tensor(xg, xn, gamma_bc, Alu.mult)
                out_t = small.tile([128, MM], f32, tag="out_t")
                nc.vector.tensor_tensor(out_t, xg, beta_bc, Alu.add)

                # store on the ACT HWDGE queue so vq prefetches on the SP
                # queue are never stuck behind an output store
                nc.scalar.dma_start(out=out_d[j * 128 : (j + 1) * 128, :], in_=out_t)

    return nc


def _get_nc():
    if "nc" not in _CACHE:
        _CACHE["nc"] = _build_nc()
    return _CACHE["nc"]


LAST_EXEC_NS = None
LAST_PROFILE_JSON = None


def kernel(q, k, vq, vk, vexp, scale, ln_gamma, ln_beta):
    import os

    from concourse import bass_utils

    nc = _get_nc()
    q = np.ascontiguousarray(np.asarray(q, dtype=np.float32))
    k = np.ascontiguousarray(np.asarray(k, dtype=np.float32))
    vq = np.ascontiguousarray(np.asarray(vq, dtype=np.float32)).reshape(B, L, MM * NN)
    vk = np.ascontiguousarray(np.asarray(vk, dtype=np.float32)).reshape(B, KK, PP * NN)
    vexp = np.ascontiguousarray(np.asarray(vexp, dtype=np.float32))
    scale_arr = np.asarray(scale, dtype=np.float32).reshape(1, 1)
    gamma_arr = np.asarray(ln_gamma, dtype=np.float32).reshape(1, DD)
    beta_arr = np.asarray(ln_beta, dtype=np.float32).reshape(1, DD)

    in_maps = [
        {
            "q": q[c],
            "k": k[c],
            "vq": vq[c],
            "vk": vk[c],
            "vexp": vexp[c],
            "scale": scale_arr,
            "ln_gamma": gamma_arr,
            "ln_beta": beta_arr,
        }
        for c in range(NCORES)
    ]
    trace = bool(os.environ.get("KERNEL_TRACE"))
    kw = {}
    if trace:
        kw = dict(trace=True, tmpdir=os.environ.get("KERNEL_TRACE_DIR") or None)
    res = bass_utils.run_bass_kernel_spmd(
        nc, in_maps, core_ids=list(range(NCORES)), **kw
    )
    if trace:
        global LAST_EXEC_NS, LAST_PROFILE_JSON
        LAST_EXEC_NS = res.exec_time_ns
        LAST_PROFILE_JSON = res.profile_json
    out = np.stack([res.results[c]["out"] for c in range(NCORES)], axis=0)
    return out.astype(np.float32)

